# revision 14
# baseline (speedup 1.0000x reference)
"""CRF loss (dense Gaussian bilateral filter) on 8 Trainium2 NeuronCores.

Math: with feats f_i (coords/ALPHA ++ I/BETA), K[i,j] = exp(-0.5*||f_i-f_j||^2),
s = K @ 1, n = (s+EPS)^-1/2, H = softmax(U), v_c = n*H_c:
    loss = n^T K n - sum_c v_c^T K v_c
(uses sum_c H_c = 1; per-batch block-diagonal K).

Sharding: cores 0-3 -> batch 0, cores 4-7 -> batch 1. Within a batch each
core owns 12 j-blocks (kernel ROWS, 1536 of the 6144-padded row space) and
the full i contraction (5888-padded columns). Row sums s_j = sum_i K_ji
fall out of the exp ACTIVATEs' accum_out for free, so there is NO device
collective: each core normalizes its own rows (n_j via ln/exp in the
preloaded natural_log_exp_and_others table set), forms W = [n, n*H_c] for
its rows, and accumulates the 5-channel partial filter output
Y_part[c, i] = sum_{j own} W_cj K_ji for ALL i via E-stationary matmuls
into a single persistent PSUM bank ([128 i-part, 5*46] accumulated over
all 552 chunk matmuls). The host sums the 4 per-core partials per batch
and does the final (tiny) loss reduction in f64.

Pipeline per block: PE dots (lhsT own-j [42,128], rhs fall [42,1536]
chunks, 3-way-split bf16, j-side -0.5|f|^2 folded into rows 39-41 so the
exp bias is uniform) -> ScalarE exp PSUM->SBUF with accum_out row-sum
partials -> (per 4-block group) ln/exp -> n -> DVE W build -> pass-B
matmuls in arrears (lag 4) interleaved with later blocks' dots.
"""

import numpy as np
import ml_dtypes

import concourse.bass as bass
import concourse.bacc as bacc
import concourse.tile as tile
import concourse.mybir as mybir
import concourse.bass_utils as bass_utils
from concourse.hw_specs import get_activation_tables

ALPHA = 5.0
BETA = 5.0
EPS = 1e-20

B = 2
C = 4
XD = YD = ZD = 18
N = XD * YD * ZD          # 5832
NI = 5888                 # i padded to 46*128
NIB = 46                  # i blocks of 128
NJO = 1536                # own j rows per core (12*128)
NJB = 12                  # own j blocks
NR = 42                   # contraction rows (39 + 3 j-side bias rows)
NEG = -120.0              # pad bias => exp -> 0
CH = [(0, 1536), (1536, 1536), (3072, 1536), (4608, 1280)]  # i chunks

F32 = mybir.dt.float32
BF16 = mybir.dt.bfloat16

TRACE = False
LAST_RESULT = None

_compiled = {}


def _build():
    nc = bacc.Bacc("TRN2", target_bir_lowering=False, debug=False, num_devices=8)

    fall = nc.dram_tensor("fall", [NR, NI], BF16, kind="ExternalInput")
    fown = nc.dram_tensor("fown", [NR, NJO], BF16, kind="ExternalInput")
    hm = nc.dram_tensor("hm", [128, 5 * NJB], F32, kind="ExternalInput")
    yt = nc.dram_tensor("yt", [128, 5 * NIB], F32, kind="ExternalOutput")
    sown = nc.dram_tensor("sown", [128, NJB], F32, kind="ExternalOutput")

    with tile.TileContext(nc) as tc:
        with (
            tc.tile_pool(name="const", bufs=1) as cp,
            tc.tile_pool(name="epool", bufs=1) as ep,
            tc.tile_pool(name="ypsum", bufs=1, space="PSUM") as yp,
            tc.tile_pool(name="wrmp", bufs=1, space="PSUM") as wp,
        ):
            fall_sb = cp.tile([NR, NI], BF16)
            fown_sb = cp.tile([NR, NJO], BF16)
            hm_sb = cp.tile([128, 5 * NJB], F32)
            eps_sb = cp.tile([128, 1], F32)
            sacc_sb = cp.tile([128, 4 * NJB], F32)
            sown_sb = cp.tile([128, NJB], F32)
            ln_sb = cp.tile([128, NJB], F32)
            n_sb = cp.tile([128, NJB], F32)
            w_sb = cp.tile([128, 5 * NJB], BF16)
            yt_sb = cp.tile([128, 5 * NIB], F32)
            scr_sb = cp.tile([128, 3072], BF16)
            wrm_sb = cp.tile([128, 512], BF16)
            e_sb = ep.tile([128, NJB * NI], BF16)

            for a in range(0, NI, 992):
                b = min(a + 992, NI)
                nc.sync.dma_start(fall_sb[:, a:b], fall[:, a:b])
            nc.sync.dma_start(fown_sb[:], fown[:])
            nc.sync.dma_start(hm_sb[:], hm[:])
            nc.vector.memset(eps_sb[:], EPS)
            nc.vector.memset(wrm_sb[:], 0.0)

            # Preload the table set holding BOTH Exp and Ln so no ACT table
            # switches land on the critical path mid-kernel.
            _tabs = list(get_activation_tables("gen3"))
            _nlx = _tabs.index("natural_log_exp_and_others")
            nc.scalar.add_instruction(
                mybir.InstLoadActFuncSet(
                    name=f"I-{nc.next_id()}", act_func_set_id=_nlx
                )
            )

            # ---- tiny PE warmup: 2 matmuls lift the p-state off LOW without
            # serializing real work behind a long warmup burst.
            wps = wp.tile([128, 512], F32)
            for i in range(2):
                nc.tensor.matmul(
                    wps[:, :],
                    wrm_sb[:, 0:128],
                    wrm_sb[:, :],
                    start=(i == 0),
                    stop=(i == 1),
                )

            # Persistent pass-B accumulator: one PSUM bank, memset once,
            # 552 accumulating matmuls share it (any start=True would clear
            # the whole bank's has_written bits -> accumulate always).
            yt_ps = yp.tile([128, 5 * NIB], F32)
            nc.vector.memset(yt_ps[:, :], 0.0)

            # Pass-B matmuls are drained from a FIFO, <=16 after each dot
            # chunk, so the PE stays continuously fed (p-state ramp) and
            # ScalarE never starves behind a long yt burst.
            pending = []

            def drain(k):
                for _ in range(min(k, len(pending))):
                    m, g = pending.pop(0)
                    nc.tensor.matmul(
                        yt_ps[:, 5 * g : 5 * (g + 1)],
                        e_sb[:, NI * m + 128 * g : NI * m + 128 * (g + 1)],
                        w_sb[:, 5 * m : 5 * (m + 1)],
                        start=False,
                        stop=(m == NJB - 1 and g == NIB - 1),
                        skip_group_check=True,
                    )

            # n-groups: pairs, then singletons for the last two blocks to
            # shorten the arrears tail.
            groups = {1: (0, 2), 3: (2, 2), 5: (4, 2), 7: (6, 2),
                      9: (8, 2), 10: (10, 1), 11: (11, 1)}

            def emit_group(g0, gw):
                nc.scalar.activation(
                    ln_sb[:, g0 : g0 + gw],
                    sown_sb[:, g0 : g0 + gw],
                    mybir.ActivationFunctionType.Ln,
                    bias=eps_sb[:, 0:1],
                    scale=1.0,
                )
                nc.scalar.activation(
                    n_sb[:, g0 : g0 + gw],
                    ln_sb[:, g0 : g0 + gw],
                    mybir.ActivationFunctionType.Exp,
                    scale=-0.5,
                )
                for mm in range(g0, g0 + gw):
                    nc.vector.tensor_scalar_mul(
                        w_sb[:, 5 * mm : 5 * (mm + 1)],
                        hm_sb[:, 5 * mm : 5 * (mm + 1)],
                        n_sb[:, mm : mm + 1],
                    )
                    pending.extend((mm, g) for g in range(NIB))

            due = None
            with tc.tile_pool(name="dotp", bufs=2, space="PSUM") as dotp:
                for m in range(NJB):
                    lw = fown_sb[:, 128 * m : 128 * (m + 1)]
                    for ci, (c0, cw) in enumerate(CH):
                        dps = dotp.tile([128, 1536], F32, tag="dot")
                        for a in range(0, cw, 512):
                            bw = min(512, cw - a)
                            nc.tensor.matmul(
                                dps[:, a : a + bw],
                                lw,
                                fall_sb[:, c0 + a : c0 + a + bw],
                                start=True,
                                stop=True,
                            )
                            # fill the dot's PSUM-write latency window with
                            # a few pass-B matmuls
                            drain(5)
                        # row-sum partials: even chunks ride the ACT
                        # accumulator, odd chunks go to DVE, so neither
                        # engine carries the whole reduction.
                        nc.scalar.activation(
                            e_sb[:, NI * m + c0 : NI * m + c0 + cw],
                            dps[:, 0:cw],
                            mybir.ActivationFunctionType.Exp,
                            scale=1.0,
                            accum_out=(
                                None if ci % 2 else
                                sacc_sb[:, 4 * m + ci : 4 * m + ci + 1]
                            ),
                        )
                        if ci % 2:
                            echunk = e_sb[:, NI * m + c0 : NI * m + c0 + cw]
                            nc.vector.scalar_tensor_tensor(
                                scr_sb[:, 0:cw],
                                echunk,
                                1.0,
                                echunk,
                                op0=mybir.AluOpType.mult,
                                op1=mybir.AluOpType.bypass,
                                accum_out=sacc_sb[:, 4 * m + ci : 4 * m + ci + 1],
                            )
                        drain(2)
                        # deferred n-group: emitted one chunk into the next
                        # block so the row-sum chain never stalls ScalarE's
                        # in-order queue
                        if ci == 1 and due is not None:
                            emit_group(*due)
                            due = None
                    nc.vector.tensor_reduce(
                        sown_sb[:, m : m + 1],
                        sacc_sb[:, 4 * m : 4 * m + 4],
                        axis=mybir.AxisListType.X,
                        op=mybir.AluOpType.add,
                    )
                    if m in groups:
                        if m >= NJB - 2:
                            emit_group(*groups[m])
                        else:
                            due = groups[m]
                drain(len(pending))

            nc.vector.tensor_copy(yt_sb[:, :], yt_ps[:, :])
            nc.sync.dma_start(yt[:, :], yt_sb[:, :])
            nc.sync.dma_start(sown[:, :], sown_sb[:, :])

    nc.compile()
    return nc


def _split3(a):
    """3-way bf16 split: a ~ h + m + l to ~24 mantissa bits."""
    bf = ml_dtypes.bfloat16
    h = a.astype(bf)
    r1 = a - h.astype(np.float32)
    m = r1.astype(bf)
    l = (r1 - m.astype(np.float32)).astype(bf)
    return h, m, l


def kernel(I, U):
    global LAST_RESULT
    if "nc" not in _compiled:
        _compiled["nc"] = _build()
    nc = _compiled["nc"]

    I = np.asarray(I, np.float32)
    U = np.asarray(U, np.float32)

    g = np.arange(XD, dtype=np.float32)
    gx, gy, gz = np.meshgrid(g, g, g, indexing="ij")
    coords = np.stack([gx, gy, gz], 0).reshape(3, N)

    bf = ml_dtypes.bfloat16
    in_maps = []
    host = []  # per batch: (H1 [4,N] f64)
    for b in range(B):
        feats = np.concatenate(
            [coords / ALPHA, I[b].reshape(3, N) / BETA], 0
        ).astype(np.float32)  # [6, N]
        sq = (feats.astype(np.float64) ** 2).sum(0)  # [N] f64
        shalf = (-0.5 * sq).astype(np.float32)
        fh, fm, fl = _split3(feats)
        s1, s2, s3 = _split3(shalf)
        one = np.ones((1, N), bf)

        # i-side (rhs, shared by the batch's 4 cores): 42 rows
        fall_np = np.zeros((NR, NI), bf)
        fall_np[:, :N] = np.concatenate(
            [fh, fm, fh, fl, fh, fm, s1[None], s2[None], s3[None],
             one, one, one], 0
        )
        fall_np[36, N:] = bf(NEG)  # invalid i columns -> exp ~ 0

        uf = U[b].reshape(C, N).astype(np.float64)
        uf = uf - uf.max(0, keepdims=True)
        e = np.exp(uf)
        H1 = e / e.sum(0, keepdims=True)  # [C, N] f64
        host.append(H1)

        for r in range(4):
            gj = NJO * r + np.arange(NJO)
            valid = gj < N
            gjv = gj[valid]
            fown_np = np.zeros((NR, NJO), bf)
            fown_np[:, valid] = np.concatenate(
                [fh[:, gjv], fh[:, gjv], fm[:, gjv], fh[:, gjv],
                 fl[:, gjv], fm[:, gjv],
                 np.ones((3, gjv.size), bf),
                 s1[None, gjv], s2[None, gjv], s3[None, gjv]], 0
            )
            # invalid j rows: all-zero dot + NEG bias via row 39 vs ones
            fown_np[39, ~valid] = bf(NEG)

            hm_np = np.zeros((128, 5 * NJB), np.float32)
            h5 = np.zeros((5, NJO), np.float32)
            h5[0, valid] = 1.0
            h5[1:, valid] = H1[:, gjv].astype(np.float32)
            # hm[p, 5m+t] = h5[t, 128m+p]
            hm_np[:, :] = h5.reshape(5, NJB, 128).transpose(2, 1, 0).reshape(128, 5 * NJB)

            in_maps.append({"fall": fall_np, "fown": fown_np, "hm": hm_np})

    res = bass_utils.run_bass_kernel_spmd(
        nc, in_maps, core_ids=list(range(8)), trace=TRACE
    )
    LAST_RESULT = res

    loss = 0.0
    for b in range(B):
        H1 = host[b]
        ytsum = np.zeros((128, 5 * NIB), np.float64)
        s_all = np.zeros(4 * NJO, np.float64)
        for r in range(4):
            k = 4 * b + r
            ytsum += res.results[k]["yt"].astype(np.float64)
            sc = res.results[k]["sown"].astype(np.float64)  # [128, 12]
            s_all[NJO * r : NJO * (r + 1)] = sc.T.reshape(NJO)
        # Y[t, 128g+p] = ytsum[p, 5g+t]
        Y = ytsum.reshape(128, NIB, 5).transpose(2, 1, 0).reshape(5, NI)[:, :N]
        s = s_all[:N]
        n = 1.0 / np.sqrt(s + EPS)
        loss += (n * Y[0]).sum()
        for c in range(C):
            loss -= (n * H1[c] * Y[1 + c]).sum()
    return np.float32(loss)


# revision 17
# speedup vs baseline: 1.0177x; 1.0177x over previous
"""CRF loss (dense Gaussian bilateral filter) on 8 Trainium2 NeuronCores.

Math: with feats f_i (coords/ALPHA ++ I/BETA), K[i,j] = exp(-0.5*||f_i-f_j||^2),
s = K @ 1, n = (s+EPS)^-1/2, H = softmax(U), v_c = n*H_c:
    loss = n^T K n - sum_c v_c^T K v_c
(uses sum_c H_c = 1; per-batch block-diagonal K).

Sharding: cores 0-3 -> batch 0, cores 4-7 -> batch 1. Within a batch each
core owns 12 j-blocks (kernel ROWS, 1536 of the 6144-padded row space) and
the full i contraction (5888-padded columns). Row sums s_j = sum_i K_ji
fall out of the exp ACTIVATEs' accum_out for free, so there is NO device
collective: each core normalizes its own rows (n_j via ln/exp in the
preloaded natural_log_exp_and_others table set), forms W = [n, n*H_c] for
its rows, and accumulates the 5-channel partial filter output
Y_part[c, i] = sum_{j own} W_cj K_ji for ALL i via E-stationary matmuls
into a single persistent PSUM bank ([128 i-part, 5*46] accumulated over
all 552 chunk matmuls). The host sums the 4 per-core partials per batch
and does the final (tiny) loss reduction in f64.

Pipeline per block: PE dots (lhsT own-j [42,128], rhs fall [42,1536]
chunks, 3-way-split bf16, j-side -0.5|f|^2 folded into rows 39-41 so the
exp bias is uniform) -> ScalarE exp PSUM->SBUF with accum_out row-sum
partials -> (per 4-block group) ln/exp -> n -> DVE W build -> pass-B
matmuls in arrears (lag 4) interleaved with later blocks' dots.
"""

import numpy as np
import ml_dtypes

import concourse.bass as bass
import concourse.bacc as bacc
import concourse.tile as tile
import concourse.mybir as mybir
import concourse.bass_utils as bass_utils
from concourse.hw_specs import get_activation_tables

ALPHA = 5.0
BETA = 5.0
EPS = 1e-20

B = 2
C = 4
XD = YD = ZD = 18
N = XD * YD * ZD          # 5832
NI = 5888                 # i padded to 46*128
NIB = 46                  # i blocks of 128
NJO = 1536                # own j rows per core (12*128)
NJB = 12                  # own j blocks
NR = 42                   # contraction rows (39 + 3 j-side bias rows)
NEG = -120.0              # pad bias => exp -> 0
CH = [(0, 1536), (1536, 1536), (3072, 1536), (4608, 1280)]  # i chunks

F32 = mybir.dt.float32
BF16 = mybir.dt.bfloat16

TRACE = False
LAST_RESULT = None

_compiled = {}


def _build():
    nc = bacc.Bacc("TRN2", target_bir_lowering=False, debug=False, num_devices=8)

    fall = nc.dram_tensor("fall", [NR, NI], BF16, kind="ExternalInput")
    fown = nc.dram_tensor("fown", [NR, NJO], BF16, kind="ExternalInput")
    hm = nc.dram_tensor("hm", [128, 5 * NJB], F32, kind="ExternalInput")
    yt = nc.dram_tensor("yt", [128, 5 * NIB], F32, kind="ExternalOutput")
    sown = nc.dram_tensor("sown", [128, NJB], F32, kind="ExternalOutput")

    with tile.TileContext(nc) as tc:
        with (
            tc.tile_pool(name="const", bufs=1) as cp,
            tc.tile_pool(name="epool", bufs=1) as ep,
            tc.tile_pool(name="ypsum", bufs=1, space="PSUM") as yp,
            tc.tile_pool(name="wrmp", bufs=1, space="PSUM") as wp,
        ):
            fall_sb = cp.tile([NR, NI], BF16)
            fown_sb = cp.tile([NR, NJO], BF16)
            hm_sb = cp.tile([128, 5 * NJB], F32)
            eps_sb = cp.tile([128, 1], F32)
            sacc_sb = cp.tile([128, 4 * NJB], F32)
            sown_sb = cp.tile([128, NJB], F32)
            ln_sb = cp.tile([128, NJB], F32)
            n_sb = cp.tile([128, NJB], F32)
            w_sb = cp.tile([128, 5 * NJB], BF16)
            yt_sb = cp.tile([128, 5 * NIB], F32)
            scr_sb = cp.tile([128, 3072], BF16)
            wrm_sb = cp.tile([128, 512], BF16)
            e_sb = ep.tile([128, NJB * NI], BF16)

            nc.sync.dma_start(fall_sb[:, 0:2944], fall[:, 0:2944])
            nc.sync.dma_start(fall_sb[:, 2944:NI], fall[:, 2944:NI])
            nc.sync.dma_start(fown_sb[:], fown[:])
            nc.sync.dma_start(hm_sb[:], hm[:])
            nc.vector.memset(eps_sb[:], EPS)
            nc.vector.memset(wrm_sb[:], 0.0)

            # Preload the table set holding BOTH Exp and Ln so no ACT table
            # switches land on the critical path mid-kernel.
            _tabs = list(get_activation_tables("gen3"))
            _nlx = _tabs.index("natural_log_exp_and_others")
            nc.scalar.add_instruction(
                mybir.InstLoadActFuncSet(
                    name=f"I-{nc.next_id()}", act_func_set_id=_nlx
                )
            )

            # ---- tiny PE warmup: 2 matmuls lift the p-state off LOW without
            # serializing real work behind a long warmup burst.
            wps = wp.tile([128, 512], F32)
            for i in range(2):
                nc.tensor.matmul(
                    wps[:, :],
                    wrm_sb[:, 0:128],
                    wrm_sb[:, :],
                    start=(i == 0),
                    stop=(i == 1),
                )

            # Persistent pass-B accumulator: one PSUM bank, memset once,
            # 552 accumulating matmuls share it (any start=True would clear
            # the whole bank's has_written bits -> accumulate always).
            yt_ps = yp.tile([128, 5 * NIB], F32)
            nc.vector.memset(yt_ps[:, :], 0.0)

            # Pass-B matmuls are drained from a FIFO, <=16 after each dot
            # chunk, so the PE stays continuously fed (p-state ramp) and
            # ScalarE never starves behind a long yt burst.
            pending = []

            def drain(k):
                for _ in range(min(k, len(pending))):
                    m, g = pending.pop(0)
                    nc.tensor.matmul(
                        yt_ps[:, 5 * g : 5 * (g + 1)],
                        e_sb[:, NI * m + 128 * g : NI * m + 128 * (g + 1)],
                        w_sb[:, 5 * m : 5 * (m + 1)],
                        start=False,
                        stop=(m == NJB - 1 and g == NIB - 1),
                        skip_group_check=True,
                    )

            # n-groups: pairs, then singletons for the last two blocks to
            # shorten the arrears tail.
            groups = {1: (0, 2), 3: (2, 2), 5: (4, 2), 7: (6, 2),
                      9: (8, 2), 10: (10, 1), 11: (11, 1)}

            def emit_group(g0, gw):
                nc.scalar.activation(
                    ln_sb[:, g0 : g0 + gw],
                    sown_sb[:, g0 : g0 + gw],
                    mybir.ActivationFunctionType.Ln,
                    bias=eps_sb[:, 0:1],
                    scale=1.0,
                )
                nc.scalar.activation(
                    n_sb[:, g0 : g0 + gw],
                    ln_sb[:, g0 : g0 + gw],
                    mybir.ActivationFunctionType.Exp,
                    scale=-0.5,
                )
                for mm in range(g0, g0 + gw):
                    nc.vector.tensor_scalar_mul(
                        w_sb[:, 5 * mm : 5 * (mm + 1)],
                        hm_sb[:, 5 * mm : 5 * (mm + 1)],
                        n_sb[:, mm : mm + 1],
                    )
                    pending.extend((mm, g) for g in range(NIB))

            due = None
            with tc.tile_pool(name="dotp", bufs=2, space="PSUM") as dotp:
                for m in range(NJB):
                    lw = fown_sb[:, 128 * m : 128 * (m + 1)]
                    for ci, (c0, cw) in enumerate(CH):
                        dps = dotp.tile([128, 1536], F32, tag="dot")
                        for a in range(0, cw, 512):
                            bw = min(512, cw - a)
                            nc.tensor.matmul(
                                dps[:, a : a + bw],
                                lw,
                                fall_sb[:, c0 + a : c0 + a + bw],
                                start=True,
                                stop=True,
                            )
                        # row-sum partials: even chunks ride the ACT
                        # accumulator, odd chunks go to DVE, so neither
                        # engine carries the whole reduction.
                        nc.scalar.activation(
                            e_sb[:, NI * m + c0 : NI * m + c0 + cw],
                            dps[:, 0:cw],
                            mybir.ActivationFunctionType.Exp,
                            scale=1.0,
                            accum_out=(
                                None if ci % 2 else
                                sacc_sb[:, 4 * m + ci : 4 * m + ci + 1]
                            ),
                        )
                        if ci % 2:
                            echunk = e_sb[:, NI * m + c0 : NI * m + c0 + cw]
                            nc.vector.scalar_tensor_tensor(
                                scr_sb[:, 0:cw],
                                echunk,
                                1.0,
                                echunk,
                                op0=mybir.AluOpType.mult,
                                op1=mybir.AluOpType.bypass,
                                accum_out=sacc_sb[:, 4 * m + ci : 4 * m + ci + 1],
                            )
                        drain(16)
                        # deferred n-group: emitted one chunk into the next
                        # block so the row-sum chain never stalls ScalarE's
                        # in-order queue
                        if ci == 1 and due is not None:
                            emit_group(*due)
                            due = None
                    nc.vector.tensor_reduce(
                        sown_sb[:, m : m + 1],
                        sacc_sb[:, 4 * m : 4 * m + 4],
                        axis=mybir.AxisListType.X,
                        op=mybir.AluOpType.add,
                    )
                    if m in groups:
                        if m >= NJB - 2:
                            emit_group(*groups[m])
                        else:
                            due = groups[m]
                drain(len(pending))

            nc.vector.tensor_copy(yt_sb[:, :], yt_ps[:, :])
            nc.sync.dma_start(yt[:, :], yt_sb[:, :])
            nc.sync.dma_start(sown[:, :], sown_sb[:, :])

    nc.compile()
    return nc


def _split3(a):
    """3-way bf16 split: a ~ h + m + l to ~24 mantissa bits."""
    bf = ml_dtypes.bfloat16
    h = a.astype(bf)
    r1 = a - h.astype(np.float32)
    m = r1.astype(bf)
    l = (r1 - m.astype(np.float32)).astype(bf)
    return h, m, l


def kernel(I, U):
    global LAST_RESULT
    if "nc" not in _compiled:
        _compiled["nc"] = _build()
    nc = _compiled["nc"]

    I = np.asarray(I, np.float32)
    U = np.asarray(U, np.float32)

    g = np.arange(XD, dtype=np.float32)
    gx, gy, gz = np.meshgrid(g, g, g, indexing="ij")
    coords = np.stack([gx, gy, gz], 0).reshape(3, N)

    bf = ml_dtypes.bfloat16
    in_maps = []
    host = []  # per batch: (H1 [4,N] f64)
    for b in range(B):
        feats = np.concatenate(
            [coords / ALPHA, I[b].reshape(3, N) / BETA], 0
        ).astype(np.float32)  # [6, N]
        sq = (feats.astype(np.float64) ** 2).sum(0)  # [N] f64
        shalf = (-0.5 * sq).astype(np.float32)
        fh, fm, fl = _split3(feats)
        s1, s2, s3 = _split3(shalf)
        one = np.ones((1, N), bf)

        # i-side (rhs, shared by the batch's 4 cores): 42 rows
        fall_np = np.zeros((NR, NI), bf)
        fall_np[:, :N] = np.concatenate(
            [fh, fm, fh, fl, fh, fm, s1[None], s2[None], s3[None],
             one, one, one], 0
        )
        fall_np[36, N:] = bf(NEG)  # invalid i columns -> exp ~ 0

        uf = U[b].reshape(C, N).astype(np.float64)
        uf = uf - uf.max(0, keepdims=True)
        e = np.exp(uf)
        H1 = e / e.sum(0, keepdims=True)  # [C, N] f64
        host.append(H1)

        for r in range(4):
            gj = NJO * r + np.arange(NJO)
            valid = gj < N
            gjv = gj[valid]
            fown_np = np.zeros((NR, NJO), bf)
            fown_np[:, valid] = np.concatenate(
                [fh[:, gjv], fh[:, gjv], fm[:, gjv], fh[:, gjv],
                 fl[:, gjv], fm[:, gjv],
                 np.ones((3, gjv.size), bf),
                 s1[None, gjv], s2[None, gjv], s3[None, gjv]], 0
            )
            # invalid j rows: all-zero dot + NEG bias via row 39 vs ones
            fown_np[39, ~valid] = bf(NEG)

            hm_np = np.zeros((128, 5 * NJB), np.float32)
            h5 = np.zeros((5, NJO), np.float32)
            h5[0, valid] = 1.0
            h5[1:, valid] = H1[:, gjv].astype(np.float32)
            # hm[p, 5m+t] = h5[t, 128m+p]
            hm_np[:, :] = h5.reshape(5, NJB, 128).transpose(2, 1, 0).reshape(128, 5 * NJB)

            in_maps.append({"fall": fall_np, "fown": fown_np, "hm": hm_np})

    res = bass_utils.run_bass_kernel_spmd(
        nc, in_maps, core_ids=list(range(8)), trace=TRACE
    )
    LAST_RESULT = res

    loss = 0.0
    for b in range(B):
        H1 = host[b]
        ytsum = np.zeros((128, 5 * NIB), np.float64)
        s_all = np.zeros(4 * NJO, np.float64)
        for r in range(4):
            k = 4 * b + r
            ytsum += res.results[k]["yt"].astype(np.float64)
            sc = res.results[k]["sown"].astype(np.float64)  # [128, 12]
            s_all[NJO * r : NJO * (r + 1)] = sc.T.reshape(NJO)
        # Y[t, 128g+p] = ytsum[p, 5g+t]
        Y = ytsum.reshape(128, NIB, 5).transpose(2, 1, 0).reshape(5, NI)[:, :N]
        s = s_all[:N]
        n = 1.0 / np.sqrt(s + EPS)
        loss += (n * Y[0]).sum()
        for c in range(C):
            loss -= (n * H1[c] * Y[1 + c]).sum()
    return np.float32(loss)


# revision 21
# speedup vs baseline: 1.0808x; 1.0619x over previous
"""CRF loss (dense Gaussian bilateral filter) on 8 Trainium2 NeuronCores.

Math: with feats f_i (coords/ALPHA ++ I/BETA), K[i,j] = exp(-0.5*||f_i-f_j||^2),
s = K @ 1, n = (s+EPS)^-1/2, H = softmax(U), v_c = n*H_c:
    loss = n^T K n - sum_c v_c^T K v_c
(uses sum_c H_c = 1; per-batch block-diagonal K).

Sharding: cores 0-3 -> batch 0, cores 4-7 -> batch 1. Within a batch each
core owns 12 j-blocks (kernel ROWS, 1536 of the 6144-padded row space) and
the full i contraction (5888-padded columns). Row sums s_j = sum_i K_ji
fall out of the exp ACTIVATEs' accum_out for free, so there is NO device
collective: each core normalizes its own rows (n_j via ln/exp in the
preloaded natural_log_exp_and_others table set), forms W = [n, n*H_c] for
its rows, and accumulates the 5-channel partial filter output
Y_part[c, i] = sum_{j own} W_cj K_ji for ALL i via E-stationary matmuls
into a single persistent PSUM bank ([128 i-part, 5*46] accumulated over
all 552 chunk matmuls). The host sums the 4 per-core partials per batch
and does the final (tiny) loss reduction in f64.

Pipeline per block: PE dots (lhsT own-j [42,128], rhs fall [42,1536]
chunks, 3-way-split bf16, j-side -0.5|f|^2 folded into rows 39-41 so the
exp bias is uniform) -> ScalarE exp PSUM->SBUF with accum_out row-sum
partials -> (per 4-block group) ln/exp -> n -> DVE W build -> pass-B
matmuls in arrears (lag 4) interleaved with later blocks' dots.
"""

import numpy as np
import ml_dtypes

import concourse.bass as bass
import concourse.bacc as bacc
import concourse.tile as tile
import concourse.mybir as mybir
import concourse.bass_utils as bass_utils
from concourse.hw_specs import get_activation_tables

ALPHA = 5.0
BETA = 5.0
EPS = 1e-20

B = 2
C = 4
XD = YD = ZD = 18
N = XD * YD * ZD          # 5832
NI = 5888                 # i padded to 46*128
NIB = 46                  # i blocks of 128
NJO = 1536                # own j rows per core (12*128)
NJB = 12                  # own j blocks
NR = 42                   # contraction rows (39 + 3 j-side bias rows)
NEG = -120.0              # pad bias => exp -> 0
# i chunks: 1024-wide (2 PSUM banks) so three dot buffers fit -> the
# PE/ScalarE pipeline is latency-decoupled instead of sem-bound
CH = [(0, 1024), (1024, 1024), (2048, 1024), (3072, 1024),
      (4096, 1024), (5120, 768)]
NCH = len(CH)

F32 = mybir.dt.float32
BF16 = mybir.dt.bfloat16

TRACE = False
LAST_RESULT = None

_compiled = {}


def _build():
    nc = bacc.Bacc("TRN2", target_bir_lowering=False, debug=False, num_devices=8)

    fall = nc.dram_tensor("fall", [NR, NI], BF16, kind="ExternalInput")
    fown = nc.dram_tensor("fown", [NR, NJO], BF16, kind="ExternalInput")
    hm = nc.dram_tensor("hm", [128, 5 * NJB], F32, kind="ExternalInput")
    yt = nc.dram_tensor("yt", [128, 5 * NIB], F32, kind="ExternalOutput")
    sown = nc.dram_tensor("sown", [128, NJB], F32, kind="ExternalOutput")

    with tile.TileContext(nc) as tc:
        with (
            tc.tile_pool(name="const", bufs=1) as cp,
            tc.tile_pool(name="epool", bufs=1) as ep,
            tc.tile_pool(name="ypsum", bufs=1, space="PSUM") as yp,
            tc.tile_pool(name="wrmp", bufs=1, space="PSUM") as wp,
        ):
            fall_sb = cp.tile([NR, NI], BF16)
            fown_sb = cp.tile([NR, NJO], BF16)
            hm_sb = cp.tile([128, 5 * NJB], F32)
            eps_sb = cp.tile([128, 1], F32)
            sacc_sb = cp.tile([128, NCH * NJB], F32)
            sown_sb = cp.tile([128, NJB], F32)
            ln_sb = cp.tile([128, NJB], F32)
            n_sb = cp.tile([128, NJB], F32)
            w_sb = cp.tile([128, 5 * NJB], BF16)
            yt_sb = cp.tile([128, 5 * NIB], F32)
            scr_sb = cp.tile([128, 3072], BF16)
            wrm_sb = cp.tile([128, 512], BF16)
            e_sb = ep.tile([128, NJB * NI], BF16)

            # first-use order: block 0 needs fown + fall[0:1024] immediately
            nc.sync.dma_start(fall_sb[:, 0:1024], fall[:, 0:1024])
            nc.sync.dma_start(fown_sb[:], fown[:])
            nc.sync.dma_start(hm_sb[:], hm[:])
            nc.sync.dma_start(fall_sb[:, 1024:3072], fall[:, 1024:3072])
            nc.sync.dma_start(fall_sb[:, 3072:NI], fall[:, 3072:NI])
            nc.vector.memset(eps_sb[:], EPS)
            nc.vector.memset(wrm_sb[:], 0.0)

            # Preload the table set holding BOTH Exp and Ln so no ACT table
            # switches land on the critical path mid-kernel.
            _tabs = list(get_activation_tables("gen3"))
            _nlx = _tabs.index("natural_log_exp_and_others")
            nc.scalar.add_instruction(
                mybir.InstLoadActFuncSet(
                    name=f"I-{nc.next_id()}", act_func_set_id=_nlx
                )
            )

            # ---- tiny PE warmup: 2 matmuls lift the p-state off LOW without
            # serializing real work behind a long warmup burst.
            wps = wp.tile([128, 512], F32)
            for i in range(2):
                nc.tensor.matmul(
                    wps[:, :],
                    wrm_sb[:, 0:128],
                    wrm_sb[:, :],
                    start=(i == 0),
                    stop=(i == 1),
                )

            # Persistent pass-B accumulator: one PSUM bank, memset once,
            # 552 accumulating matmuls share it (any start=True would clear
            # the whole bank's has_written bits -> accumulate always).
            yt_ps = yp.tile([128, 5 * NIB], F32)
            nc.vector.memset(yt_ps[:, :], 0.0)

            # Pass-B matmuls are drained from a FIFO, <=16 after each dot
            # chunk, so the PE stays continuously fed (p-state ramp) and
            # ScalarE never starves behind a long yt burst.
            pending = []

            def drain(k):
                for _ in range(min(k, len(pending))):
                    m, g = pending.pop(0)
                    nc.tensor.matmul(
                        yt_ps[:, 5 * g : 5 * (g + 1)],
                        e_sb[:, NI * m + 128 * g : NI * m + 128 * (g + 1)],
                        w_sb[:, 5 * m : 5 * (m + 1)],
                        start=False,
                        stop=(m == NJB - 1 and g == NIB - 1),
                        skip_group_check=True,
                    )

            # n-groups: pairs, then singletons for the last two blocks to
            # shorten the arrears tail.
            groups = {1: (0, 2), 3: (2, 2), 5: (4, 2), 7: (6, 2),
                      9: (8, 2), 10: (10, 1), 11: (11, 1)}

            def emit_group(g0, gw):
                nc.scalar.activation(
                    ln_sb[:, g0 : g0 + gw],
                    sown_sb[:, g0 : g0 + gw],
                    mybir.ActivationFunctionType.Ln,
                    bias=eps_sb[:, 0:1],
                    scale=1.0,
                )
                nc.scalar.activation(
                    n_sb[:, g0 : g0 + gw],
                    ln_sb[:, g0 : g0 + gw],
                    mybir.ActivationFunctionType.Exp,
                    scale=-0.5,
                )
                for mm in range(g0, g0 + gw):
                    nc.vector.tensor_scalar_mul(
                        w_sb[:, 5 * mm : 5 * (mm + 1)],
                        hm_sb[:, 5 * mm : 5 * (mm + 1)],
                        n_sb[:, mm : mm + 1],
                    )
                    pending.extend((mm, g) for g in range(NIB))

            due = None
            with tc.tile_pool(name="dotp", bufs=3, space="PSUM") as dotp:
                for m in range(NJB):
                    lw = fown_sb[:, 128 * m : 128 * (m + 1)]
                    for ci, (c0, cw) in enumerate(CH):
                        dps = dotp.tile([128, 1024], F32, tag="dot")
                        for a in range(0, cw, 512):
                            bw = min(512, cw - a)
                            nc.tensor.matmul(
                                dps[:, a : a + bw],
                                lw,
                                fall_sb[:, c0 + a : c0 + a + bw],
                                start=True,
                                stop=True,
                            )
                        # row-sum partials: first and last chunk ride the
                        # ACT accumulator (keeps the block-end n chain off
                        # DVE), middle chunks go to DVE.
                        on_act = ci in (0, NCH - 1)
                        nc.scalar.activation(
                            e_sb[:, NI * m + c0 : NI * m + c0 + cw],
                            dps[:, 0:cw],
                            mybir.ActivationFunctionType.Exp,
                            scale=1.0,
                            accum_out=(
                                sacc_sb[:, NCH * m + ci : NCH * m + ci + 1]
                                if on_act else None
                            ),
                        )
                        if not on_act:
                            echunk = e_sb[:, NI * m + c0 : NI * m + c0 + cw]
                            nc.vector.scalar_tensor_tensor(
                                scr_sb[:, 0:cw],
                                echunk,
                                1.0,
                                echunk,
                                op0=mybir.AluOpType.mult,
                                op1=mybir.AluOpType.bypass,
                                accum_out=sacc_sb[:, NCH * m + ci : NCH * m + ci + 1],
                            )
                        drain(8)
                        # deferred n-group: emitted one chunk into the next
                        # block so the row-sum chain never stalls ScalarE's
                        # in-order queue
                        if ci == 1 and due is not None:
                            emit_group(*due)
                            due = None
                    nc.vector.tensor_reduce(
                        sown_sb[:, m : m + 1],
                        sacc_sb[:, NCH * m : NCH * m + NCH],
                        axis=mybir.AxisListType.X,
                        op=mybir.AluOpType.add,
                    )
                    if m in groups:
                        if m >= NJB - 2:
                            emit_group(*groups[m])
                        else:
                            due = groups[m]
                drain(len(pending))

            nc.vector.tensor_copy(yt_sb[:, :], yt_ps[:, :])
            nc.sync.dma_start(yt[:, :], yt_sb[:, :])
            nc.sync.dma_start(sown[:, :], sown_sb[:, :])

    nc.compile()
    return nc


def _split3(a):
    """3-way bf16 split: a ~ h + m + l to ~24 mantissa bits."""
    bf = ml_dtypes.bfloat16
    h = a.astype(bf)
    r1 = a - h.astype(np.float32)
    m = r1.astype(bf)
    l = (r1 - m.astype(np.float32)).astype(bf)
    return h, m, l


def kernel(I, U):
    global LAST_RESULT
    if "nc" not in _compiled:
        _compiled["nc"] = _build()
    nc = _compiled["nc"]

    I = np.asarray(I, np.float32)
    U = np.asarray(U, np.float32)

    g = np.arange(XD, dtype=np.float32)
    gx, gy, gz = np.meshgrid(g, g, g, indexing="ij")
    coords = np.stack([gx, gy, gz], 0).reshape(3, N)

    bf = ml_dtypes.bfloat16
    in_maps = []
    host = []  # per batch: (H1 [4,N] f64)
    for b in range(B):
        feats = np.concatenate(
            [coords / ALPHA, I[b].reshape(3, N) / BETA], 0
        ).astype(np.float32)  # [6, N]
        sq = (feats.astype(np.float64) ** 2).sum(0)  # [N] f64
        shalf = (-0.5 * sq).astype(np.float32)
        fh, fm, fl = _split3(feats)
        s1, s2, s3 = _split3(shalf)
        one = np.ones((1, N), bf)

        # i-side (rhs, shared by the batch's 4 cores): 42 rows
        fall_np = np.zeros((NR, NI), bf)
        fall_np[:, :N] = np.concatenate(
            [fh, fm, fh, fl, fh, fm, s1[None], s2[None], s3[None],
             one, one, one], 0
        )
        fall_np[36, N:] = bf(NEG)  # invalid i columns -> exp ~ 0

        uf = U[b].reshape(C, N).astype(np.float64)
        uf = uf - uf.max(0, keepdims=True)
        e = np.exp(uf)
        H1 = e / e.sum(0, keepdims=True)  # [C, N] f64
        host.append(H1)

        for r in range(4):
            gj = NJO * r + np.arange(NJO)
            valid = gj < N
            gjv = gj[valid]
            fown_np = np.zeros((NR, NJO), bf)
            fown_np[:, valid] = np.concatenate(
                [fh[:, gjv], fh[:, gjv], fm[:, gjv], fh[:, gjv],
                 fl[:, gjv], fm[:, gjv],
                 np.ones((3, gjv.size), bf),
                 s1[None, gjv], s2[None, gjv], s3[None, gjv]], 0
            )
            # invalid j rows: all-zero dot + NEG bias via row 39 vs ones
            fown_np[39, ~valid] = bf(NEG)

            hm_np = np.zeros((128, 5 * NJB), np.float32)
            h5 = np.zeros((5, NJO), np.float32)
            h5[0, valid] = 1.0
            h5[1:, valid] = H1[:, gjv].astype(np.float32)
            # hm[p, 5m+t] = h5[t, 128m+p]
            hm_np[:, :] = h5.reshape(5, NJB, 128).transpose(2, 1, 0).reshape(128, 5 * NJB)

            in_maps.append({"fall": fall_np, "fown": fown_np, "hm": hm_np})

    res = bass_utils.run_bass_kernel_spmd(
        nc, in_maps, core_ids=list(range(8)), trace=TRACE
    )
    LAST_RESULT = res

    loss = 0.0
    for b in range(B):
        H1 = host[b]
        ytsum = np.zeros((128, 5 * NIB), np.float64)
        s_all = np.zeros(4 * NJO, np.float64)
        for r in range(4):
            k = 4 * b + r
            ytsum += res.results[k]["yt"].astype(np.float64)
            sc = res.results[k]["sown"].astype(np.float64)  # [128, 12]
            s_all[NJO * r : NJO * (r + 1)] = sc.T.reshape(NJO)
        # Y[t, 128g+p] = ytsum[p, 5g+t]
        Y = ytsum.reshape(128, NIB, 5).transpose(2, 1, 0).reshape(5, NI)[:, :N]
        s = s_all[:N]
        n = 1.0 / np.sqrt(s + EPS)
        loss += (n * Y[0]).sum()
        for c in range(C):
            loss -= (n * H1[c] * Y[1 + c]).sum()
    return np.float32(loss)
